# revision 38
# baseline (speedup 1.0000x reference)
"""Cross-attention (global, batch-flattened K/V) Trainium2 kernel, v5.

Problem: emb [16, 4096, 64]; two cross-attention halves:
  out_l2u = cross(q=emb[:8],  kv=emb[8:])   -> rows 0..7
  out_u2l = cross(q=emb[8:],  kv=emb[:8])   -> rows 8..15
cross(): q/k/v proj (64->512), s = einsum('bnc,nd->bcd', q, kflat),
InstanceNorm over (CH, B*CH) plane per b, softmax over d, ctx = a @ vflat^T,
out = ctx @ Wout.

Sharding: 16 independent (cross, q-batch) instances, 2 per core.
Cores 0-3: q from lower half (kv = upper), cores 4-7: q from upper
(kv = lower). No collectives; weights replicated.

Key insight: the score matrix is RANK-64 (all projections factor
through the 64-channel embedding), so both big GEMMs contract through
64-dim intermediates:
  sT[d,:] for kv batch db = Wk^T @ (emb_db^T @ q)      (m = emb^T q: [64,512])
  outT = sum_db (wvo_db)^T @ emb_db^T,  wvo_db = (Wv @ aT_db / den) @ Wout
The (CH x D) score plane is still materialized (transposed, bf16) for
InstanceNorm + softmax, drained with fused stats (ssum on DVE
tensor_scalar accum, ssq on DVE tensor_tensor_reduce; exp on scalar).
emb is cast to bf16 once, stored pair-packed+padded [N,128] in DRAM;
natural-order loads feed the m phase, XBAR DMA-transposed loads feed
the output phase. q is built straight from the fp32 input via PE
transposes so nothing waits on the DRAM staging. m phase is software-
pipelined (m for batch db+1 issued before sT of batch db).
"""

import numpy as np
import concourse.bass as bass
import concourse.mybir as mybir
import concourse.tile as tile
from concourse import bacc
from concourse.bass_utils import run_bass_kernel_spmd

dt = mybir.dt
AF = mybir.ActivationFunctionType
ALU = mybir.AluOpType

B = 8            # batches per half
N = 4096         # sequence length
C = 64           # embedding channels
CH = 512         # num_heads * C
NB = N // 128    # 32 n-blocks
CB = CH // 128   # 4 c-blocks
D = B * CH       # 4096 flattened kv dim
EPS = 1e-5
BF = dt.bfloat16
PLANE = float(CH * D)  # InstanceNorm plane size per instance

_nc = None


def _build():
    nc = bacc.Bacc("TRN2", target_bir_lowering=False, debug=False, num_devices=8)

    embq = nc.declare_dram_parameter("embq", [2, N, C], dt.float32, isOutput=False)
    embkv = nc.declare_dram_parameter("embkv", [B, N, C], dt.float32, isOutput=False)
    Wq_d = nc.declare_dram_parameter("Wq", [C, CH], dt.float32, isOutput=False)
    Wk_d = nc.declare_dram_parameter("Wk", [C, CH], dt.float32, isOutput=False)
    Wv_d = nc.declare_dram_parameter("Wv", [C, CH], dt.float32, isOutput=False)
    Wout_d = nc.declare_dram_parameter("Wout", [CH, C], dt.float32, isOutput=False)
    ident_d = nc.declare_dram_parameter("ident", [128, 128], dt.float32, isOutput=False)
    ones_d = nc.declare_dram_parameter("ones", [128, 128], dt.float32, isOutput=False)
    out_d = nc.declare_dram_parameter("out", [2, C, N], dt.float32, isOutput=True)

    # bf16 kv emb, pair-packed: slot k = kv batches (2k | 2k+1) in cols
    # (0:64 | 64:128).
    emb_bf = nc.dram_tensor("emb_bf", [4, N, 128], BF)

    with tile.TileContext(nc) as tc:
        with (
            tc.tile_pool(name="const", bufs=1) as constp,
            tc.tile_pool(name="io", bufs=2) as iop,
            tc.tile_pool(name="res", bufs=1) as resp,
            tc.tile_pool(name="stream", bufs=2) as streamp,
            tc.tile_pool(name="small", bufs=1) as smallp,
            tc.tile_pool(name="ps", bufs=6, space="PSUM") as psp,
        ):
            # ---- constants ----
            ident = constp.tile([128, 128], dt.float32, tag="ident")
            nc.sync.dma_start(ident[:], ident_d[:])
            ident_bf = constp.tile([128, 128], BF, tag="ident_bf")
            nc.vector.tensor_copy(out=ident_bf[:], in_=ident[:])
            ones_f = iop.tile([128, 128], dt.float32, tag="wst")
            nc.sync.dma_start(ones_f[:], ones_d[:])
            ones_r = constp.tile([128, 128], dt.float32r, tag="ones_r")
            nc.vector.tensor_copy(out=ones_r[:], in_=ones_f[:])
            onescol = constp.tile([128, 1], BF, tag="onescol")
            nc.vector.tensor_copy(out=onescol[:], in_=ones_f[:, 0:1])

            w_bf = {}
            for name, wd in (("Wq", Wq_d), ("Wv", Wv_d)):
                wst = iop.tile([C, CH], dt.float32, tag="wst")
                nc.sync.dma_start(wst[:], wd[:])
                wb = constp.tile([C, CH], BF, tag=f"{name}_bf")
                nc.vector.tensor_copy(out=wb[:], in_=wst[:])
                if name == "Wv":
                    wv_f32 = wst
                w_bf[name] = wb
            Wq_b = w_bf["Wq"]
            # Wk duplicated on both partition halves so the sT matmul can
            # read m for the odd batch of a pair at base partition 64
            wk2st = constp.tile([128, CH], dt.float32, tag="wk2st")
            nc.sync.dma_start(wk2st[0:C, :], Wk_d[:])
            nc.sync.dma_start(wk2st[C:128, :], Wk_d[:])
            Wk2_b = constp.tile([128, CH], BF, tag="Wk2_bf")
            nc.vector.tensor_copy(out=Wk2_b[:], in_=wk2st[:])

            # WvT [128(ch sub), CB, 64(c')] via 4 fp32 PE transposes
            WvT_b = constp.tile([128, CB, C], BF, tag="WvT_bf")
            ptw = psp.tile([128, 512], dt.float32, tag="pp")
            for k in range(CB):
                nc.tensor.transpose(
                    ptw[:, k * 128:k * 128 + C],
                    wv_f32[:, k * 128:(k + 1) * 128],
                    ident[0:C, 0:C],
                )
            for k in range(CB):
                nc.vector.tensor_copy(
                    out=WvT_b[:, k, :], in_=ptw[:, k * 128:k * 128 + C]
                )

            wost = iop.tile([128, CB, C], dt.float32, tag="wst")
            nc.sync.dma_start(
                wost[:], Wout_d[:].rearrange("(cb p) c -> p cb c", p=128)
            )
            Wout_b = constp.tile([128, CB, C], BF, tag="Wout_bf")
            nc.vector.tensor_copy(out=Wout_b[:], in_=wost[:])

            # ---- preamble: cast kv emb to bf16, pair-packed. The half
            # tiles stay RESIDENT in SBUF (the m phase reads them
            # directly); the DRAM copy is only for the XBAR-transposed
            # output-phase loads and is issued lazily from the scalar
            # queue. Rows of half tile (k, h): n = h*2048 + p*16 + nb. ----
            lb_tiles = []

            def to_bf(slot):
                for h in range(2):
                    lb = iop.tile([128, 16, 128], BF, tag="ldb", bufs=8)
                    for half in range(2):
                        lt = iop.tile([128, 16, C], dt.float32, tag="ld")
                        nc.sync.dma_start(
                            lt[:],
                            embkv[2 * slot + half,
                                  h * 2048:(h + 1) * 2048, :].rearrange(
                                "(p nb) c -> p nb c", p=128
                            ),
                        )
                        nc.vector.tensor_copy(
                            out=lb[:, :, half * C:(half + 1) * C], in_=lt[:]
                        )
                    nc.scalar.dma_start(
                        emb_bf[slot, h * 2048:(h + 1) * 2048, :].rearrange(
                            "(p nb) c -> p nb c", p=128
                        ),
                        lb[:],
                    )
                    lb_tiles.append(lb)

            # ---- persistent SBUF tensors ----
            q_sb = resp.tile([128, NB, CH], BF, tag="q")      # 32KB/part

            ssum = smallp.tile([128, 2, NB], dt.float32, tag="ssum")
            ssq = smallp.tile([128, 2, NB], dt.float32, tag="ssq")
            stats2 = smallp.tile([128, 2, 8], dt.float32, tag="stats2")
            invden2 = smallp.tile([128, 2, CB], dt.float32, tag="invden2")

            def build_q(inst):
                """q[n,ch] straight from fp32 embq, in the same half-block
                layout as the lb tiles: block j = h*16+nb holds rows
                n = h*2048 + p*16 + nb."""
                embt_q = streamp.tile([C, N], BF, tag="embt", bufs=1)
                for h in range(2):
                    lt = iop.tile([128, 16, C], dt.float32, tag="ld")
                    nc.sync.dma_start(
                        lt[:],
                        embq[inst, h * 2048:(h + 1) * 2048, :].rearrange(
                            "(p nb) c -> p nb c", p=128
                        ),
                    )
                    lbq = iop.tile([128, 16, C], BF, tag="lbq")
                    nc.vector.tensor_copy(out=lbq[:], in_=lt[:])
                    for grp in range(4):
                        pb = psp.tile([128, 512], BF, tag="ppb", bufs=2)
                        for j4 in range(4):
                            nb = grp * 4 + j4
                            nc.tensor.transpose(
                                pb[0:C, j4 * 128:(j4 + 1) * 128],
                                lbq[:, nb, :],
                                ident_bf[:],
                            )
                        j0 = h * 16 + grp * 4
                        nc.vector.tensor_copy(
                            out=embt_q[:, j0 * 128:(j0 + 4) * 128],
                            in_=pb[0:C, :],
                        )
                for j in range(NB):
                    pt = psp.tile([128, 512], dt.float32, tag="pp")
                    nc.tensor.matmul(
                        pt[:],
                        embt_q[:, j * 128:(j + 1) * 128],
                        Wq_b[:],
                        start=True,
                        stop=True,
                    )
                    nc.vector.tensor_copy(out=q_sb[:, j, :], in_=pt[:])

            def m_phase(pair):
                """m for BOTH batches of the pair in one go: lhsT packs the
                two batches' channels on the 128 partitions -> [128, CH].
                Reads the resident lb half tiles."""
                pm = psp.tile([128, 512], dt.float32, tag="pp", name="pm")
                for h in range(2):
                    lb = lb_tiles[2 * pair + h]
                    for nb in range(16):
                        nc.tensor.matmul(
                            pm[:],
                            lb[:, nb, :],
                            q_sb[:, h * 16 + nb, :],
                            start=(h == 0 and nb == 0),
                            stop=(h == 1 and nb == 15),
                        )
                m_sb = streamp.tile([128, CH], BF, tag="msb")
                nc.vector.tensor_copy(out=m_sb[:], in_=pm[:])
                return m_sb

            def sT_phase(inst, db, m_sb, sT):
                half = (db % 2) * C
                ps_sT = [psp.tile([128, 512], dt.float32, tag="pp",
                                  name=f"ps_sT{i}") for i in range(CB)]
                for dc in range(CB):
                    nc.tensor.matmul(
                        ps_sT[dc][:],
                        Wk2_b[half:half + C, dc * 128:(dc + 1) * 128],
                        m_sb[half:half + C, :],
                        start=True,
                        stop=True,
                    )
                for dc in range(CB):
                    kb = db * CB + dc
                    nc.scalar.activation(
                        sT[:, kb, :], ps_sT[dc][:], AF.Copy,
                        accum_out=ssum[:, inst, kb:kb + 1],
                    )
                    nc.scalar.activation(
                        ps_sT[dc][:], ps_sT[dc][:], AF.Square,
                        accum_out=ssq[:, inst, kb:kb + 1],
                    )

            def s_pass(inst, filler=None):
                """sT[d, c] = Wk^T @ (emb_db^T @ q), software-pipelined:
                m for pair k+1 issued before sT of pair k. `filler(pair)`
                lets the caller interleave scalar-engine work (prev
                instance's exp) between pairs."""
                sT = resp.tile([128, NB, CH], BF, tag="sT", bufs=2)
                prev = None
                for pair in range(4):
                    m_sb = m_phase(pair)
                    if prev is not None:
                        sT_phase(inst, 2 * prev[0], prev[1], sT)
                        sT_phase(inst, 2 * prev[0] + 1, prev[1], sT)
                    if filler is not None:
                        filler(pair)
                    prev = (pair, m_sb)
                sT_phase(inst, 2 * prev[0], prev[1], sT)
                sT_phase(inst, 2 * prev[0] + 1, prev[1], sT)
                return sT

            def exp_chunk(inst, sT, k4):
                stats = stats2[:, inst, :]
                nc.scalar.activation(
                    sT[:, k4 * 4:(k4 + 1) * 4, :],
                    sT[:, k4 * 4:(k4 + 1) * 4, :], AF.Exp,
                    bias=stats[:, 6:7], scale=stats[:, 5:6],
                )

            def stats_prep(inst, sT):
                """InstanceNorm stats -> rstd / -mu*rstd in stats2."""
                stats = stats2[:, inst, :]
                red = smallp.tile([128, 2], dt.float32, tag="red", bufs=2)
                nc.vector.tensor_reduce(
                    out=red[:, 0:1], in_=ssum[:, inst, :],
                    axis=mybir.AxisListType.X, op=ALU.add,
                )
                nc.vector.tensor_reduce(
                    out=red[:, 1:2], in_=ssq[:, inst, :],
                    axis=mybir.AxisListType.X, op=ALU.add,
                )
                red_r = smallp.tile([128, 2], dt.float32r, tag="red_r", bufs=2)
                nc.vector.tensor_copy(out=red_r[:], in_=red[:])
                ptr = psp.tile([128, 512], dt.float32, tag="pp")
                nc.tensor.matmul(
                    ptr[:, 0:2], ones_r[:], red_r[:], start=True, stop=True
                )
                nc.scalar.activation(
                    stats[:, 0:2], ptr[:, 0:2], AF.Copy, bias=0.0,
                    scale=1.0 / PLANE,
                )
                mu = stats[:, 0:1]
                ex2 = stats[:, 1:2]
                musq = stats[:, 2:3]
                var = stats[:, 3:4]
                std = stats[:, 4:5]
                rstd = stats[:, 5:6]
                nmr = stats[:, 6:7]
                nc.vector.tensor_tensor(out=musq, in0=mu, in1=mu, op=ALU.mult)
                nc.vector.tensor_tensor(out=var, in0=ex2, in1=musq,
                                        op=ALU.subtract)
                nc.vector.tensor_scalar_add(var, var, EPS)
                nc.scalar.activation(std, var, AF.Sqrt, bias=0.0)
                nc.vector.reciprocal(rstd, std)
                nc.vector.tensor_tensor(out=nmr, in0=mu, in1=rstd, op=ALU.mult)
                nc.scalar.mul(nmr, nmr, -1.0)

            def den_pass(inst, sT):
                """softmax denominator per ch -> invden2[:, inst, :]."""
                ps_den = psp.tile([128, 512], dt.float32, tag="pp",
                                  name="ps_den")
                for kb in range(NB):
                    nc.tensor.matmul(
                        ps_den[0:1, :], onescol[:], sT[:, kb, :],
                        start=(kb == 0), stop=(kb == NB - 1),
                    )
                den_sb = smallp.tile([1, CH], BF, tag="den_sb", bufs=2)
                nc.scalar.activation(den_sb[:], ps_den[0:1, :], AF.Copy)
                ptd = psp.tile([128, 512], dt.float32, tag="pp", name="ptd")
                for cb in range(CB):
                    nc.tensor.matmul(
                        ptd[:, cb:cb + 1],
                        den_sb[0:1, cb * 128:(cb + 1) * 128],
                        onescol[0:1, 0:1],
                        start=True, stop=True,
                    )
                denT = smallp.tile([128, CB], dt.float32, tag="denT", bufs=2)
                nc.scalar.activation(denT[:], ptd[:, 0:CB], AF.Copy)
                nc.vector.reciprocal(invden2[:, inst, :], denT[:])

            etp_tiles = []

            def load_etp():
                for k in range(4):
                    etp = streamp.tile([128, N], BF, tag="etp", bufs=4)
                    nc.sync.dma_start_transpose(etp[:], emb_bf[k])
                    etp_tiles.append(etp)

            def wva_phase(inst, db, sT):
                pwa = psp.tile([128, 512], dt.float32, tag="pp", name="pwa")
                for chb in range(CB):
                    for j in range(CB):
                        nc.tensor.matmul(
                            pwa[:, chb * C:(chb + 1) * C],
                            sT[:, db * CB + j, chb * 128:(chb + 1) * 128],
                            WvT_b[:, j, :],
                            start=(j == 0),
                            stop=(j == CB - 1),
                        )
                wva = streamp.tile([128, CB, C], BF, tag="wva")
                for chb in range(CB):
                    nc.vector.tensor_scalar_mul(
                        wva[:, chb, :],
                        pwa[:, chb * C:(chb + 1) * C],
                        invden2[:, inst, chb:chb + 1],
                    )
                return wva

            def ctx_out(inst, sT):
                """outT = sum_db (wvo_db)^T @ emb_db^T."""
                pwo = psp.tile([128, 512], dt.float32, tag="pp", name="pwo")

                def wvo_phase(db, wva):
                    for chb in range(CB):
                        nc.tensor.matmul(
                            pwo[0:C, db * C:(db + 1) * C],
                            wva[:, chb, :],
                            Wout_b[:, chb, :],
                            start=(chb == 0),
                            stop=(chb == CB - 1),
                        )

                prev = None
                for db in range(B):
                    wva = wva_phase(inst, db, sT)
                    if prev is not None:
                        wvo_phase(prev[0], prev[1])
                    prev = (db, wva)
                wvo_phase(prev[0], prev[1])
                # pair-stack wvo: [128(2 batches' c'), 4(pair), 64]
                wvo2 = streamp.tile([128, CB, C], BF, tag="wvo2")
                for db in range(B):
                    nc.vector.tensor_copy(
                        out=wvo2[(db % 2) * C:(db % 2 + 1) * C, db // 2, :],
                        in_=pwo[0:C, db * C:(db + 1) * C],
                    )
                # outT[c, n] = sum_pairs wvo_pair^T @ embT_pair
                for g in range(8):
                    pout = psp.tile([128, 512], dt.float32, tag="pp",
                                    name="pout")
                    for k in range(4):
                        nc.tensor.matmul(
                            pout[0:C, :],
                            wvo2[:, k, :],
                            etp_tiles[k][:, g * 512:(g + 1) * 512],
                            start=(k == 0),
                            stop=(k == 3),
                        )
                    ot = streamp.tile([C, 512], dt.float32, tag="ot")
                    nc.vector.tensor_copy(out=ot[:], in_=pout[0:C, :])
                    nc.sync.dma_start(
                        out_d[inst, :, g * 512:(g + 1) * 512], ot[:]
                    )

            # ================= schedule =================
            to_bf(0)
            build_q(0)
            for k in range(1, 4):
                to_bf(k)
            sT0 = s_pass(0)
            build_q(1)
            stats_prep(0, sT0)

            def fill0(pair):
                exp_chunk(0, sT0, 2 * pair)
                exp_chunk(0, sT0, 2 * pair + 1)

            sT1 = s_pass(1, filler=fill0)
            load_etp()
            den_pass(0, sT0)
            stats_prep(1, sT1)
            for k4 in range(8):
                exp_chunk(1, sT1, k4)
            ctx_out(0, sT0)
            den_pass(1, sT1)
            ctx_out(1, sT1)

    nc.compile()
    return nc


def _get_nc():
    global _nc
    if _nc is None:
        _nc = _build()
    return _nc


def kernel(emb, Wq, Wk, Wv, Wout):
    emb = np.ascontiguousarray(emb, dtype=np.float32)
    Wq = np.ascontiguousarray(Wq, dtype=np.float32)
    Wk = np.ascontiguousarray(Wk, dtype=np.float32)
    Wv = np.ascontiguousarray(Wv, dtype=np.float32)
    Wout = np.ascontiguousarray(Wout, dtype=np.float32)
    emb_l, emb_u = emb[:B], emb[B:]
    ident = np.eye(128, dtype=np.float32)
    ones = np.ones((128, 128), dtype=np.float32)

    in_maps = []
    for core in range(8):
        if core < 4:
            qb, kvb = emb_l[2 * core:2 * core + 2], emb_u
        else:
            j = core - 4
            qb, kvb = emb_u[2 * j:2 * j + 2], emb_l
        in_maps.append({
            "embq": np.ascontiguousarray(qb), "embkv": np.ascontiguousarray(kvb),
            "Wq": Wq, "Wk": Wk, "Wv": Wv, "Wout": Wout, "ident": ident,
            "ones": ones,
        })

    res = run_bass_kernel_spmd(_get_nc(), in_maps, list(range(8))).results

    out = np.empty((2 * B, N, C), np.float32)
    for core in range(8):
        o = res[core]["out"].transpose(0, 2, 1)  # [2, C, N] -> [2, N, C]
        if core < 4:
            out[2 * core:2 * core + 2] = o
        else:
            j = core - 4
            out[B + 2 * j:B + 2 * j + 2] = o
    return out
